# revision 13
# baseline (speedup 1.0000x reference)
"""BarrierNet Trainium2 kernel: MLP (6->128->128x2 branches->heads) + closed-form QP.

Data-parallel over 8 cores (16384 samples each). Host pre-shards and
pre-transposes: xt [6,NS] bf16, and packs all weights/biases/QP constants into
two blob tensors (1 bf16 + 1 fp32) loaded with single DMAs. Per core:
  - MLP in transposed layout (hidden on partitions, batch free), chunks of
    2048; each layer = 4 bf16 N=512 matmuls into a 4-bank PSUM group drained
    by ONE fused bias+tanh ACT op (fp32 PSUM -> bf16 SBUF).
  - The ACT engine is the bottleneck (1 elem/lane/cycle), so ~7 of the 41
    chunk-drains are offloaded to the otherwise-idle DVE as a polynomial
    tanh approximation in bf16 (deg-2 in x^2 for the narrow-range layers,
    a Pade(3,2) rational for fc1). Offloaded chunks sit at phase starts so
    their longer latency hides under the ACT drains of the same phase.
  - Heads: per 128-sample block the activation chunk is the STATIONARY
    matmul operand and the tiny head weight [128,3]/[128,2] is the moving
    one, so head output lands directly in the QP sample-grid layout in PSUM.
  - QP runs batched fp32 in a [128, j] sample grid split into asymmetric
    halves (12288 + 4096 samples): the big half's QP runs on the Pool engine
    under the fcm drains; only the small half's QP is a tail.
"""
import sys

sys.path.insert(0, "/opt/trn_rl_repo")

import numpy as np
import ml_dtypes

import concourse.bacc as bacc
import concourse.bass as bass
import concourse.mybir as mybir
import concourse.tile as tile
from concourse import bass_utils

FP = mybir.dt.float32
BF = mybir.dt.bfloat16
AF = mybir.ActivationFunctionType
OP = mybir.AluOpType
BF_NP = ml_dtypes.bfloat16

N_CORES = 8
B = 131072
NS = B // N_CORES          # samples per core
SC = 2048                  # super-chunk (one PSUM group span)
H = 128
NF = 6

# QP halves: (sample offset, jh = samples per grid partition)
HALVES = [(0, 96), (12288, 32)]
QCB = [0, 17 * 96]         # qc column base per half
BFP_W = 17 * 96 + 17 * 32 + 5

# blob_bf columns (bf16)
C_W21, C_W22, C_WM1, C_WM2, C_W1, C_WH1, C_WH2 = 0, 128, 256, 384, 512, 640, 643
BBF_W = 645

# tanh approximations (minimax fits of tanh(x)/x in t=x^2, bf16-validated)
P2_WIDE = (0.95591543, -0.20820148, 0.02276596)    # |x|<=2.0 (fc21/fc22)
P2_NARROW = (0.99716053, -0.30797275, 0.07279147)  # |x|<=1.0 (fcm1/fcm2)

_cache = {}


def build(ns=NS):
    nit = ns // SC
    nc = bacc.Bacc("TRN2", target_bir_lowering=False, debug=False)

    x_d = nc.dram_tensor("x", [ns, NF], FP, kind="ExternalInput")
    xt_d = nc.dram_tensor("xt", [NF, ns], BF, kind="ExternalInput")
    bbf_d = nc.dram_tensor("bbf", [H, BBF_W], BF, kind="ExternalInput")
    bfp_d = nc.dram_tensor("bfp", [H, BFP_W], FP, kind="ExternalInput")
    u_d = nc.dram_tensor("u", [ns, 3], FP, kind="ExternalOutput")

    with tile.TileContext(nc) as tc:
        with (
            tc.tile_pool(name="const", bufs=1) as cpool,
            tc.tile_pool(name="act", bufs=2) as apool,
            tc.tile_pool(name="psum", bufs=2, space="PSUM") as ppool,
            tc.tile_pool(name="qp", bufs=2) as qpool,
        ):
            bbf = cpool.tile([H, BBF_W], BF, tag="bbf", name="bbf")
            nc.sync.dma_start(bbf[:], bbf_d[:])
            bfp = cpool.tile([H, BFP_W], FP, tag="bfp", name="bfp")
            nc.sync.dma_start(bfp[:], bfp_d[:])

            def bias(k):
                return bfp[:, QCB[1] + 17 * 32 + k : QCB[1] + 17 * 32 + k + 1]

            hT_all = cpool.tile([H, ns], BF, tag="hT_all", name="hT_all")
            x21a = cpool.tile([H, ns], BF, tag="x21a", name="x21a")
            x22a = cpool.tile([H, ns], BF, tag="x22a", name="x22a")
            x21b = cpool.tile([H, ns], BF, tag="x21b", name="x21b")
            x22b = cpool.tile([H, ns], BF, tag="x22b", name="x22b")

            # QP scratch: per-half persistent values + one shared transient
            # region (each engine runs its QP ops in-order, so reuse is safe).
            pers = [cpool.tile([H, 7 * jh], FP, tag=f"pers{h}", name=f"pers{h}")
                    for h, (_, jh) in enumerate(HALVES)]
            scr = cpool.tile([H, 1536], FP, tag="qscr", name="qscr")
            # DVE tanh-offload scratch (bf16)
            txb = cpool.tile([H, SC], BF, tag="txb", name="txb")
            tt = cpool.tile([H, SC], BF, tag="tt", name="tt")
            th = cpool.tile([H, SC], BF, tag="th", name="th")

            V = nc.vector
            G = nc.gpsimd

            def S(lo, w):
                return scr[:, lo : lo + w]

            # Dummy sigmoid up front: the compiler then picks the activation
            # table set containing BOTH sigmoid and tanh -> one table load.
            V.memset(S(1520, 8), 0.0)
            nc.scalar.activation(S(1528, 8), S(1520, 8), AF.Sigmoid)

            # ---------------- QP (sample-grid layout, fp32, batched) --------
            def qp_pre(hh):
                """x-side preamble: needs only x_d -- runs on idle DVE."""
                off, jh = HALVES[hh]
                J3, J6 = 3 * jh, 6 * jh
                qc0 = QCB[hh]
                xg = qpool.tile([128, 6 * 96], FP, tag="xg", name="xg")
                nc.sync.dma_start(xg[:, 0:J6],
                    x_d[off : off + 128 * jh, :].rearrange(
                        "(p j) f -> p (j f)", p=128))
                xgv = xg[:, 0:J6].rearrange("p (j g e) -> p e g j", g=3, e=2)
                x0 = S(0, J6)
                x0v = x0.rearrange("p (e g j) -> p e g j", e=2, g=3)
                qsv = bfp[:, qc0 : qc0 + J6].rearrange(
                    "p (e g j) -> p e g j", e=2, g=3)
                qmv = bfp[:, qc0 + J6 : qc0 + 2 * J6].rearrange(
                    "p (e g j) -> p e g j", e=2, g=3)
                V.tensor_mul(x0v, xgv, qsv)
                V.tensor_add(x0v, x0v, qmv)
                # Persistent values are pre-scaled so the post chain is pure
                # tensor-tensor ops (the Pool engine's ISA has no
                # tensor-scalar): d3x4 = 4d^3 = -G, bar16 = 16*barrier,
                # bd4 = 4*barrier_dot, Ls12 = Lf2b, rg = 1/GG.
                dd, vv = x0[:, 0:J3], x0[:, J3:J6]
                d3 = pers[hh][:, 0:J3]
                d2 = S(J6, J3); V.tensor_mul(d2, dd, dd)
                V.tensor_mul(d3, d2, dd)
                V.tensor_scalar(d3, d3, 4.0, None, OP.mult)    # d3x4
                d4 = S(0, J3); V.tensor_mul(d4, d2, d2)        # over dd
                v2 = S(J6 + J3, J3); V.tensor_mul(v2, vv, vv)
                dv = S(J6 + 2 * J3, J3); V.tensor_mul(dv, d3, vv)   # 4 d^3 v
                dw = S(J3, J3); V.tensor_mul(dw, d2, v2)       # over vv
                d6 = S(J6, J3); V.tensor_mul(d6, d3, d3)       # 16 d^6, over d2
                g6 = S(J6 + 3 * J3, jh)

                def a3(t, k):
                    return t[:, k * jh : (k + 1) * jh]

                def sum3(t, r, bias_const=None):
                    V.tensor_add(r, a3(t, 0), a3(t, 1))
                    if bias_const is None:
                        V.tensor_add(r, r, a3(t, 2))
                    else:
                        V.scalar_tensor_tensor(r, r, bias_const, a3(t, 2),
                                               OP.add, OP.add)

                bar = pers[hh][:, J3 : J3 + jh]
                bd = pers[hh][:, J3 + jh : J3 + 2 * jh]
                Ls = pers[hh][:, J3 + 2 * jh : J3 + 3 * jh]
                sum3(d4, bar, -2401.0)                 # barrier
                V.tensor_scalar(bar, bar, 16.0, None, OP.mult)  # 16*barrier
                sum3(dv, bd)                           # barrier_dot
                V.tensor_scalar(bd, bd, 4.0, None, OP.mult)     # 4*dot
                sum3(dw, Ls)                           # Lf2b / 12
                V.tensor_scalar(Ls, Ls, 12.0, None, OP.mult)    # Lf2b
                sum3(d6, g6)                           # GG
                V.reciprocal(pers[hh][:, J3 + 3 * jh : J3 + 4 * jh], g6)

            # Stationary-operand views matching the QP grid: grid partition p
            # holds samples off + p*jh + j, so head block j takes columns
            # {off + i*jh + j : i=0..127} (stride jh) of the activation tiles.
            def hview(t, hh):
                off, jh = HALVES[hh]
                return t[:, off : off + 128 * jh].rearrange(
                    "p (i j) -> p j i", j=jh)

            def heads31(hh, psH):
                v = hview(x21b, hh)
                for j in range(HALVES[hh][1]):
                    nc.tensor.matmul(psH[:, 5 * j : 5 * j + 3], v[:, j, :],
                                     bbf[:, C_WH1 : C_WH1 + 3],
                                     start=True, stop=True)

            def heads32(hh, psH):
                v = hview(x22b, hh)
                for j in range(HALVES[hh][1]):
                    nc.tensor.matmul(psH[:, 5 * j + 3 : 5 * j + 5], v[:, j, :],
                                     bbf[:, C_WH2 : C_WH2 + 2],
                                     start=True, stop=True)

            # qp_post scratch offsets (units of jh)
            def qp_post_a(hh, psH, E):
                """x31-side: needs only the fcm1 branch (heads31 results)."""
                off, jh = HALVES[hh]
                J2, J3, J6 = 2 * jh, 3 * jh, 6 * jh
                qc0 = QCB[hh]
                hgv = psH[:, 0 : 5 * jh].rearrange("p (j c) -> p c j", c=5)
                x31v = S(2 * J2, J3)
                V.tensor_add(x31v.rearrange("p (c j) -> p c j", c=3),
                             hgv[:, 0:3, :],
                             bfp[:, qc0 + 2 * J6 : qc0 + 2 * J6 + J3].rearrange(
                                 "p (c j) -> p c j", c=3))
                gx = S(2 * J2 + J3, J3)
                E.tensor_mul(gx, pers[hh][:, 0:J3], x31v)
                gu = S(2 * J2 + 2 * J3, jh)
                E.tensor_add(gu, gx[:, 0:jh], gx[:, jh : 2 * jh])
                E.tensor_add(gu, gu, gx[:, 2 * jh : J3])

            def qp_post_b(hh, psH, E):
                off, jh = HALVES[hh]
                J2, J3, J6 = 2 * jh, 3 * jh, 6 * jh
                qc0 = QCB[hh]
                p_ = pers[hh]
                bar = p_[:, J3 : J3 + jh]
                bd = p_[:, J3 + jh : J3 + 2 * jh]
                Ls = p_[:, J3 + 2 * jh : J3 + 3 * jh]
                rg = p_[:, J3 + 3 * jh : J3 + 4 * jh]
                hgv = psH[:, 0 : 5 * jh].rearrange("p (j c) -> p c j", c=5)
                zs = S(0, J2)
                V.tensor_add(zs.rearrange("p (c j) -> p c j", c=2),
                             hgv[:, 3:5, :],
                             bfp[:, qc0 + 2 * J6 + J3 : qc0 + 17 * jh].rearrange(
                                 "p (c j) -> p c j", c=2))
                sg = S(J2, J2)
                nc.scalar.activation(sg, zs, AF.Sigmoid)
                x31v = S(2 * J2, J3)
                gu = S(2 * J2 + 2 * J3, jh)
                o = 2 * J2 + 2 * J3 + jh
                gxb = 2 * J2 + J3    # gx region, dead after gu
                s0t, s1t = sg[:, 0:jh], sg[:, jh:J2]
                # x32sum*bdot = 4*sigmoid_sum*bdot = ssum*bd4;
                # x32prod*bar = 16*sigmoid_prod*bar = sprod*bar16.
                ssum = S(o, jh); E.tensor_add(ssum, s0t, s1t)
                sprod = S(o + jh, jh); E.tensor_mul(sprod, s0t, s1t)
                t1 = S(o + 2 * jh, jh); E.tensor_mul(t1, ssum, bd)
                t2 = S(o + 3 * jh, jh); E.tensor_mul(t2, sprod, bar)
                qa = S(gxb, jh); E.tensor_sub(qa, gu, t1)
                qb = S(gxb + jh, jh); E.tensor_add(qb, Ls, t2)
                q = S(gxb + 2 * jh, jh); E.tensor_sub(q, qa, qb)
                E.tensor_relu(q, q)
                lam = S(o, jh); E.tensor_mul(lam, q, rg)    # over ssum
                ui = qpool.tile([128, 3 * 96], FP, tag="ui", name="ui")
                uiv = ui[:, 0:J3].rearrange("p (j c) -> p c j", c=3)
                w = S(o + jh, jh)                           # over sprod
                for a in range(3):
                    # u = lam*4d^3 - x31  (G = -4d^3, d3 holds 4d^3)
                    E.tensor_mul(w, lam, p_[:, a * jh : (a + 1) * jh])
                    E.tensor_sub(uiv[:, a, :], w,
                                 x31v[:, a * jh : (a + 1) * jh])
                nc.sync.dma_start(
                    u_d[off : off + 128 * jh, :].rearrange(
                        "(p j) c -> p (j c)", p=128),
                    ui[:, 0:J3])

            # ---------------- MLP: layer-outer phases ----------------
            def dve_tanh(ps, w, bias_t, out_sl, kind):
                """Drain a PSUM group via DVE tanh approximation (bf16).
                The copy out of PSUM folds in the per-channel bias."""
                xb, t, h = txb[:, 0:w], tt[:, 0:w], th[:, 0:w]
                V.tensor_scalar(xb, ps[:, 0:w], bias_t, None, OP.add)
                V.tensor_mul(t, xb, xb)
                if kind == "pade":
                    # x*(t+27)/(9t+27), clamp
                    V.tensor_scalar(h, t, 27.0, None, OP.add)
                    V.tensor_mul(h, h, xb)
                    V.tensor_scalar(t, t, 9.0, 27.0, OP.mult, OP.add)
                    with nc.allow_low_precision(reason="pade recip bf16"):
                        V.reciprocal(t, t)
                    V.tensor_mul(out_sl, h, t)
                else:
                    c0, c1, c2 = kind
                    V.tensor_scalar(h, t, c2, c1, OP.mult, OP.add)
                    V.tensor_mul(h, h, t)
                    V.tensor_scalar(h, h, c0, None, OP.add)
                    V.tensor_mul(out_sl, h, xb)
                V.tensor_scalar(out_sl, out_sl, 1.0, -1.0, OP.min, OP.max)

            def chunk_layer(lhsT, rhs_sl, bias_t, out_sl, w=SC, offload=None):
                ps = ppool.tile([128, SC], FP, tag="ps", name="ps")
                for m in range((w + 511) // 512):
                    mw = min(512, w - 512 * m)
                    nc.tensor.matmul(
                        ps[:, 512 * m : 512 * m + mw],
                        lhsT,
                        rhs_sl[:, 512 * m : 512 * m + mw],
                        start=True, stop=True,
                    )
                if offload is None:
                    nc.scalar.activation(out_sl, ps[:, 0:w], AF.Tanh,
                                         bias=bias_t)
                else:
                    dve_tanh(ps, w, bias_t, out_sl, offload)

            def csl(t, i, w=SC):
                return t[:, SC * i : SC * i + w]

            # fc1 phase: first chunk split 512+1536 for a faster ACT start.
            # fc1 bias is NOT folded into the matmul, so add it via the ACT
            # op (bias arg) or, for the offloaded chunk, fold into xt? --
            # offloaded fc1 chunk handles bias by a pre-add on DVE instead.
            w1 = bbf[0:NF, C_W1 : C_W1 + H]
            for i in range(nit):
                xt_c = apool.tile([NF, SC], BF, tag="xt_c", name="xt_c")
                eng = nc.gpsimd.dma_start if i % 2 == 0 else nc.sync.dma_start
                eng(xt_c[:], xt_d[:, SC * i : SC * (i + 1)])
                if i == 0:
                    chunk_layer(w1, xt_c[:, 0:512], bias(0), csl(hT_all, 0, 512),
                                w=512)
                    chunk_layer(w1, xt_c[:, 512:2048], bias(0),
                                hT_all[:, 512:2048], w=1536)
                else:
                    chunk_layer(w1, xt_c[:], bias(0), csl(hT_all, i))
            qp_pre(0)
            qp_pre(1)

            OFF21 = {0: P2_WIDE, 1: P2_WIDE}
            OFF22 = {0: P2_WIDE}
            for i in range(nit):
                chunk_layer(bbf[:, C_W21 : C_W21 + H], csl(hT_all, i), bias(1),
                            csl(x21a, i), offload=OFF21.get(i))
                chunk_layer(bbf[:, C_W22 : C_W22 + H], csl(hT_all, i), bias(2),
                            csl(x22a, i), offload=OFF22.get(i))

            OFFM1 = {0: P2_NARROW, 1: P2_NARROW}
            OFFM2 = {0: P2_NARROW}
            psH = {}
            for i in range(nit):
                chunk_layer(bbf[:, C_WM1 : C_WM1 + H], csl(x21a, i), bias(3),
                            csl(x21b, i), offload=OFFM1.get(i))
                chunk_layer(bbf[:, C_WM2 : C_WM2 + H], csl(x22a, i), bias(4),
                            csl(x22b, i), offload=OFFM2.get(i))
                if i == 5:
                    # half 0 (chunks 0-5) complete: heads + QP under the
                    # remaining drains; arithmetic on the idle Pool engine.
                    psH[0] = ppool.tile([128, SC], FP, tag="ps", name="psH0")
                    heads31(0, psH[0])
                    heads32(0, psH[0])
                    qp_post_a(0, psH[0], G)
                    qp_post_b(0, psH[0], G)
            psH[1] = ppool.tile([128, SC], FP, tag="ps", name="psH1")
            heads31(1, psH[1])
            qp_post_a(1, psH[1], V)
            heads32(1, psH[1])
            qp_post_b(1, psH[1], V)

    nc.compile()
    return nc


def _get_nc(ns=NS):
    if ns not in _cache:
        _cache[ns] = build(ns)
    return _cache[ns]


def prep_maps(inputs, ns=NS, n_cores=N_CORES):
    """Host-side shard + layout prep. Returns per-core in_maps."""
    f32 = np.float32
    g = {k: np.asarray(v) for k, v in inputs.items()}
    x = np.ascontiguousarray(g["x"], f32)
    mean = np.asarray(g["mean"], f32)
    std = np.asarray(g["std"], f32)
    obs = np.array([10.0, 0.0, 10.0, 0.0, 9.0, 0.0], f32)
    moff = mean - obs
    perm = [0, 2, 4, 1, 3, 5]  # pos-block | vel-block order
    qcs = []
    for _, jh in HALVES:
        qcs.append(np.concatenate([
            np.repeat(std[perm], jh),
            np.repeat(moff[perm], jh),
            np.repeat(np.asarray(g["fc31_b"], f32), jh),
            np.repeat(np.asarray(g["fc32_b"], f32), jh),
        ]))
    qc = np.concatenate(qcs)
    bfp = np.concatenate([
        np.broadcast_to(qc, (H, qc.size)),
        np.asarray(g["fc1_b"], f32)[:, None],
        np.asarray(g["fc21_b"], f32)[:, None],
        np.asarray(g["fc22_b"], f32)[:, None],
        np.asarray(g["fcm1_b"], f32)[:, None],
        np.asarray(g["fcm2_b"], f32)[:, None],
    ], axis=1)
    w1pad = np.zeros((H, H), f32)
    w1pad[:NF, :] = np.asarray(g["fc1_w"], f32).T
    bbf = np.concatenate([
        np.asarray(g["fc21_w"], f32).T,
        np.asarray(g["fc22_w"], f32).T,
        np.asarray(g["fcm1_w"], f32).T,
        np.asarray(g["fcm2_w"], f32).T,
        w1pad,
        np.asarray(g["fc31_w"], f32).T,
        np.asarray(g["fc32_w"], f32).T,
    ], axis=1).astype(BF_NP)

    shared = {
        "bbf": np.ascontiguousarray(bbf),
        "bfp": np.ascontiguousarray(bfp, f32),
    }
    in_maps = []
    for c in range(n_cores):
        sh = x[c * ns : (c + 1) * ns]
        m = dict(shared)
        m["x"] = np.ascontiguousarray(sh)
        m["xt"] = np.ascontiguousarray(sh.T.astype(BF_NP))
        in_maps.append(m)
    return in_maps


def kernel(**inputs):
    nc = _get_nc()
    in_maps = prep_maps(inputs)
    res = bass_utils.run_bass_kernel_spmd(nc, in_maps, core_ids=list(range(N_CORES)))
    return np.concatenate([res.results[c]["u"] for c in range(N_CORES)], axis=0)


# revision 15
# speedup vs baseline: 1.1792x; 1.1792x over previous
"""BarrierNet Trainium2 kernel: MLP (6->128->128x2 branches->heads) + closed-form QP.

Data-parallel over 8 cores (16384 samples each). Host pre-shards and
pre-transposes: xt [6,NS] bf16, and packs all weights/biases/QP constants into
two blob tensors (1 bf16 + 1 fp32) loaded with single DMAs. Per core:
  - MLP in transposed layout (hidden on partitions, batch free), chunks of
    2048; each layer = 4 bf16 N=512 matmuls into a 4-bank PSUM group drained
    by ONE fused bias+tanh ACT op (fp32 PSUM -> bf16 SBUF).
  - The ACT engine is the bottleneck (1 elem/lane/cycle), so ~7 of the 41
    chunk-drains are offloaded to the otherwise-idle DVE as a polynomial
    tanh approximation in bf16 (deg-2 in x^2 for the narrow-range layers,
    a Pade(3,2) rational for fc1). Offloaded chunks sit at phase starts so
    their longer latency hides under the ACT drains of the same phase.
  - Heads: per 128-sample block the activation chunk is the STATIONARY
    matmul operand and the tiny head weight [128,3]/[128,2] is the moving
    one, so head output lands directly in the QP sample-grid layout in PSUM.
  - QP runs batched fp32 in a [128, j] sample grid split into asymmetric
    halves (12288 + 4096 samples): the big half's QP runs on the Pool engine
    under the fcm drains; only the small half's QP is a tail.
"""
import sys

sys.path.insert(0, "/opt/trn_rl_repo")

import numpy as np
import ml_dtypes

import concourse.bacc as bacc
import concourse.bass as bass
import concourse.mybir as mybir
import concourse.tile as tile
from concourse import bass_utils

FP = mybir.dt.float32
BF = mybir.dt.bfloat16
AF = mybir.ActivationFunctionType
OP = mybir.AluOpType
BF_NP = ml_dtypes.bfloat16

N_CORES = 8
B = 131072
NS = B // N_CORES          # samples per core
SC = 2048                  # super-chunk (one PSUM group span)
H = 128
NF = 6

# QP halves: (sample offset, jh = samples per grid partition)
HALVES = [(0, 96), (12288, 32)]
QCB = [0, 17 * 96]         # qc column base per half
BFP_W = 17 * 96 + 17 * 32 + 5

# blob_bf columns (bf16)
C_W21, C_W22, C_WM1, C_WM2, C_W1, C_WH1, C_WH2 = 0, 128, 256, 384, 512, 640, 643
BBF_W = 645

# tanh approximations (minimax fits of tanh(x)/x in t=x^2, bf16-validated)
P2_WIDE = (0.95591543, -0.20820148, 0.02276596)    # |x|<=2.0 (fc21/fc22)
P2_NARROW = (0.99716053, -0.30797275, 0.07279147)  # |x|<=1.0 (fcm1/fcm2)

_cache = {}


def build(ns=NS):
    nit = ns // SC
    nc = bacc.Bacc("TRN2", target_bir_lowering=False, debug=False)

    x_d = nc.dram_tensor("x", [ns, NF], FP, kind="ExternalInput")
    xt_d = nc.dram_tensor("xt", [NF, ns], BF, kind="ExternalInput")
    bbf_d = nc.dram_tensor("bbf", [H, BBF_W], BF, kind="ExternalInput")
    bfp_d = nc.dram_tensor("bfp", [H, BFP_W], FP, kind="ExternalInput")
    u_d = nc.dram_tensor("u", [ns, 3], FP, kind="ExternalOutput")

    with tile.TileContext(nc) as tc:
        with (
            tc.tile_pool(name="const", bufs=1) as cpool,
            tc.tile_pool(name="act", bufs=2) as apool,
            tc.tile_pool(name="psum", bufs=2, space="PSUM") as ppool,
            tc.tile_pool(name="qp", bufs=2) as qpool,
            tc.tile_pool(name="xb", bufs=2) as xpool,
        ):
            bbf = cpool.tile([H, BBF_W], BF, tag="bbf", name="bbf")
            nc.sync.dma_start(bbf[:], bbf_d[:])
            bfp = cpool.tile([H, BFP_W], FP, tag="bfp", name="bfp")
            nc.sync.dma_start(bfp[:], bfp_d[:])

            def bias(k):
                return bfp[:, QCB[1] + 17 * 32 + k : QCB[1] + 17 * 32 + k + 1]

            hT_all = cpool.tile([H, ns], BF, tag="hT_all", name="hT_all")
            x21a = cpool.tile([H, ns], BF, tag="x21a", name="x21a")
            x22a = cpool.tile([H, ns], BF, tag="x22a", name="x22a")
            x22b = cpool.tile([H, ns], BF, tag="x22b", name="x22b")
            # x21b overlays hT_all: hT is fully consumed by fc21/fc22 before
            # fcm1 writes x21b (subtile deps order the overwrite per chunk).
            x21b = hT_all

            # QP scratch: per-half persistent values + one shared transient
            # region (each engine runs its QP ops in-order, so reuse is safe).
            pers = [cpool.tile([H, 7 * jh], FP, tag=f"pers{h}", name=f"pers{h}")
                    for h, (_, jh) in enumerate(HALVES)]
            scr = cpool.tile([H, 1536], FP, tag="qscr", name="qscr")
            # DVE tanh-offload scratch (bf16)
            tt = cpool.tile([H, SC], BF, tag="tt", name="tt")
            th = cpool.tile([H, SC], BF, tag="th", name="th")

            V = nc.vector
            G = nc.gpsimd

            def S(lo, w):
                return scr[:, lo : lo + w]

            # Dummy sigmoid up front: the compiler then picks the activation
            # table set containing BOTH sigmoid and tanh -> one table load.
            V.memset(S(1520, 8), 0.0)
            nc.scalar.activation(S(1528, 8), S(1520, 8), AF.Sigmoid)

            # ---------------- QP (sample-grid layout, fp32, batched) --------
            xgs = {}
            for hh, (off, jh) in enumerate(HALVES):
                xg = qpool.tile([128, 6 * 96], FP, tag="xg", name="xg")
                nc.sync.dma_start(xg[:, 0 : 6 * jh],
                    x_d[off : off + 128 * jh, :].rearrange(
                        "(p j) f -> p (j f)", p=128))
                xgs[hh] = xg

            def qp_pre(hh):
                """x-side preamble: needs only x_d -- runs on idle DVE."""
                off, jh = HALVES[hh]
                J3, J6 = 3 * jh, 6 * jh
                qc0 = QCB[hh]
                xg = xgs[hh]
                xgv = xg[:, 0:J6].rearrange("p (j g e) -> p e g j", g=3, e=2)
                x0 = S(0, J6)
                x0v = x0.rearrange("p (e g j) -> p e g j", e=2, g=3)
                qsv = bfp[:, qc0 : qc0 + J6].rearrange(
                    "p (e g j) -> p e g j", e=2, g=3)
                qmv = bfp[:, qc0 + J6 : qc0 + 2 * J6].rearrange(
                    "p (e g j) -> p e g j", e=2, g=3)
                V.tensor_mul(x0v, xgv, qsv)
                V.tensor_add(x0v, x0v, qmv)
                # Persistent values are pre-scaled so the post chain is pure
                # tensor-tensor ops (the Pool engine's ISA has no
                # tensor-scalar): d3x4 = 4d^3 = -G, bar16 = 16*barrier,
                # bd4 = 4*barrier_dot, Ls12 = Lf2b, rg = 1/GG.
                dd, vv = x0[:, 0:J3], x0[:, J3:J6]
                d3 = pers[hh][:, 0:J3]
                d2 = S(J6, J3); V.tensor_mul(d2, dd, dd)
                V.tensor_mul(d3, d2, dd)
                V.tensor_scalar(d3, d3, 4.0, None, OP.mult)    # d3x4
                d4 = S(0, J3); V.tensor_mul(d4, d2, d2)        # over dd
                v2 = S(J6 + J3, J3); V.tensor_mul(v2, vv, vv)
                dv = S(J6 + 2 * J3, J3); V.tensor_mul(dv, d3, vv)   # 4 d^3 v
                dw = S(J3, J3); V.tensor_mul(dw, d2, v2)       # over vv
                d6 = S(J6, J3); V.tensor_mul(d6, d3, d3)       # 16 d^6, over d2
                g6 = S(J6 + 3 * J3, jh)

                def a3(t, k):
                    return t[:, k * jh : (k + 1) * jh]

                def sum3(t, r, bias_const=None):
                    V.tensor_add(r, a3(t, 0), a3(t, 1))
                    if bias_const is None:
                        V.tensor_add(r, r, a3(t, 2))
                    else:
                        V.scalar_tensor_tensor(r, r, bias_const, a3(t, 2),
                                               OP.add, OP.add)

                bar = pers[hh][:, J3 : J3 + jh]
                bd = pers[hh][:, J3 + jh : J3 + 2 * jh]
                Ls = pers[hh][:, J3 + 2 * jh : J3 + 3 * jh]
                sum3(d4, bar, -2401.0)                 # barrier
                V.tensor_scalar(bar, bar, 16.0, None, OP.mult)  # 16*barrier
                sum3(dv, bd)                           # barrier_dot
                V.tensor_scalar(bd, bd, 4.0, None, OP.mult)     # 4*dot
                sum3(dw, Ls)                           # Lf2b / 12
                V.tensor_scalar(Ls, Ls, 12.0, None, OP.mult)    # Lf2b
                sum3(d6, g6)                           # GG
                V.reciprocal(pers[hh][:, J3 + 3 * jh : J3 + 4 * jh], g6)

            # Stationary-operand views matching the QP grid: grid partition p
            # holds samples off + p*jh + j, so head block j takes columns
            # {off + i*jh + j : i=0..127} (stride jh) of the activation tiles.
            def hview(t, hh):
                off, jh = HALVES[hh]
                return t[:, off : off + 128 * jh].rearrange(
                    "p (i j) -> p j i", j=jh)

            def heads31(hh, psH):
                v = hview(x21b, hh)
                for j in range(HALVES[hh][1]):
                    nc.tensor.matmul(psH[:, 5 * j : 5 * j + 3], v[:, j, :],
                                     bbf[:, C_WH1 : C_WH1 + 3],
                                     start=True, stop=True)

            def heads32(hh, psH):
                v = hview(x22b, hh)
                for j in range(HALVES[hh][1]):
                    nc.tensor.matmul(psH[:, 5 * j + 3 : 5 * j + 5], v[:, j, :],
                                     bbf[:, C_WH2 : C_WH2 + 2],
                                     start=True, stop=True)

            # qp_post scratch offsets (units of jh)
            def qp_post_a(hh, psH, E):
                """x31-side: needs only the fcm1 branch (heads31 results)."""
                off, jh = HALVES[hh]
                J2, J3, J6 = 2 * jh, 3 * jh, 6 * jh
                qc0 = QCB[hh]
                hgv = psH[:, 0 : 5 * jh].rearrange("p (j c) -> p c j", c=5)
                x31v = S(2 * J2, J3)
                V.tensor_add(x31v.rearrange("p (c j) -> p c j", c=3),
                             hgv[:, 0:3, :],
                             bfp[:, qc0 + 2 * J6 : qc0 + 2 * J6 + J3].rearrange(
                                 "p (c j) -> p c j", c=3))
                gx = S(2 * J2 + J3, J3)
                E.tensor_mul(gx, pers[hh][:, 0:J3], x31v)
                gu = S(2 * J2 + 2 * J3, jh)
                E.tensor_add(gu, gx[:, 0:jh], gx[:, jh : 2 * jh])
                E.tensor_add(gu, gu, gx[:, 2 * jh : J3])

            def qp_post_b(hh, psH, E):
                off, jh = HALVES[hh]
                J2, J3, J6 = 2 * jh, 3 * jh, 6 * jh
                qc0 = QCB[hh]
                p_ = pers[hh]
                bar = p_[:, J3 : J3 + jh]
                bd = p_[:, J3 + jh : J3 + 2 * jh]
                Ls = p_[:, J3 + 2 * jh : J3 + 3 * jh]
                rg = p_[:, J3 + 3 * jh : J3 + 4 * jh]
                hgv = psH[:, 0 : 5 * jh].rearrange("p (j c) -> p c j", c=5)
                zs = S(0, J2)
                V.tensor_add(zs.rearrange("p (c j) -> p c j", c=2),
                             hgv[:, 3:5, :],
                             bfp[:, qc0 + 2 * J6 + J3 : qc0 + 17 * jh].rearrange(
                                 "p (c j) -> p c j", c=2))
                sg = S(J2, J2)
                nc.scalar.activation(sg, zs, AF.Sigmoid)
                x31v = S(2 * J2, J3)
                gu = S(2 * J2 + 2 * J3, jh)
                o = 2 * J2 + 2 * J3 + jh
                gxb = 2 * J2 + J3    # gx region, dead after gu
                s0t, s1t = sg[:, 0:jh], sg[:, jh:J2]
                # x32sum*bdot = 4*sigmoid_sum*bdot = ssum*bd4;
                # x32prod*bar = 16*sigmoid_prod*bar = sprod*bar16.
                ssum = S(o, jh); E.tensor_add(ssum, s0t, s1t)
                sprod = S(o + jh, jh); E.tensor_mul(sprod, s0t, s1t)
                t1 = S(o + 2 * jh, jh); E.tensor_mul(t1, ssum, bd)
                t2 = S(o + 3 * jh, jh); E.tensor_mul(t2, sprod, bar)
                qa = S(gxb, jh); E.tensor_sub(qa, gu, t1)
                qb = S(gxb + jh, jh); E.tensor_add(qb, Ls, t2)
                q = S(gxb + 2 * jh, jh); E.tensor_sub(q, qa, qb)
                E.tensor_relu(q, q)
                lam = S(o, jh); E.tensor_mul(lam, q, rg)    # over ssum
                ui = qpool.tile([128, 3 * 96], FP, tag="ui", name="ui")
                uiv = ui[:, 0:J3].rearrange("p (j c) -> p c j", c=3)
                w = S(o + jh, jh)                           # over sprod
                for a in range(3):
                    # u = lam*4d^3 - x31  (G = -4d^3, d3 holds 4d^3)
                    E.tensor_mul(w, lam, p_[:, a * jh : (a + 1) * jh])
                    E.tensor_sub(uiv[:, a, :], w,
                                 x31v[:, a * jh : (a + 1) * jh])
                nc.sync.dma_start(
                    u_d[off : off + 128 * jh, :].rearrange(
                        "(p j) c -> p (j c)", p=128),
                    ui[:, 0:J3])

            # ---------------- MLP: layer-outer phases ----------------
            pending = []   # deferred DVE tanh chains (bounds xb tiles at 2)

            def flush_chain(all_=False):
                while pending and (all_ or len(pending) >= 2):
                    pending.pop(0)()

            def dve_tanh(ps, w, bias_t, out_sl, kind):
                """Drain a PSUM group via DVE tanh approximation (bf16).
                The PSUM->bf16 copy (emitted now, freeing the PSUM group
                quickly) folds in the per-channel bias; the arithmetic chain
                is deferred so later copies are not stuck behind it."""
                xb = xpool.tile([H, SC], BF, tag="txb", name="txb")
                V.tensor_scalar(xb[:, 0:w], ps[:, 0:w], bias_t, None, OP.add)

                def chain():
                    t, h, xbs = tt[:, 0:w], th[:, 0:w], xb[:, 0:w]
                    V.tensor_mul(t, xbs, xbs)
                    if kind == "pade":
                        # x*(t+27)/(9t+27), clamp
                        V.tensor_scalar(h, t, 27.0, None, OP.add)
                        V.tensor_mul(h, h, xbs)
                        V.tensor_scalar(t, t, 9.0, 27.0, OP.mult, OP.add)
                        with nc.allow_low_precision(reason="pade recip bf16"):
                            V.reciprocal(t, t)
                        V.tensor_mul(out_sl, h, t)
                    else:
                        c0, c1, c2 = kind
                        V.tensor_scalar(h, t, c2, c1, OP.mult, OP.add)
                        V.tensor_mul(h, h, t)
                        V.tensor_scalar(h, h, c0, None, OP.add)
                        V.tensor_mul(out_sl, h, xbs)
                    V.tensor_scalar(out_sl, out_sl, 1.0, -1.0, OP.min, OP.max)

                pending.append(chain)
                flush_chain()

            def chunk_layer(lhsT, rhs_sl, bias_t, out_sl, w=SC, offload=None):
                ps = ppool.tile([128, SC], FP, tag="ps", name="ps")
                for m in range((w + 511) // 512):
                    mw = min(512, w - 512 * m)
                    nc.tensor.matmul(
                        ps[:, 512 * m : 512 * m + mw],
                        lhsT,
                        rhs_sl[:, 512 * m : 512 * m + mw],
                        start=True, stop=True,
                    )
                if offload is None:
                    nc.scalar.activation(out_sl, ps[:, 0:w], AF.Tanh,
                                         bias=bias_t)
                else:
                    dve_tanh(ps, w, bias_t, out_sl, offload)

            def csl(t, i, w=SC):
                return t[:, SC * i : SC * i + w]

            # fc1 phase: first chunk split 512+1536 for a faster ACT start.
            # fc1 bias is NOT folded into the matmul, so add it via the ACT
            # op (bias arg) or, for the offloaded chunk, fold into xt? --
            # offloaded fc1 chunk handles bias by a pre-add on DVE instead.
            w1 = bbf[0:NF, C_W1 : C_W1 + H]
            for i in range(nit):
                xt_c = apool.tile([NF, SC], BF, tag="xt_c", name="xt_c")
                eng = nc.gpsimd.dma_start if i % 2 == 0 else nc.sync.dma_start
                eng(xt_c[:], xt_d[:, SC * i : SC * (i + 1)])
                if i == 0:
                    chunk_layer(w1, xt_c[:, 0:512], bias(0), csl(hT_all, 0, 512),
                                w=512)
                    chunk_layer(w1, xt_c[:, 512:2048], bias(0),
                                hT_all[:, 512:2048], w=1536)
                else:
                    chunk_layer(w1, xt_c[:], bias(0), csl(hT_all, i),
                                offload="pade" if i == 2 else None)
            flush_chain(all_=True)
            qp_pre(0)
            qp_pre(1)

            OFF21 = {0: P2_WIDE, 3: P2_WIDE}
            OFF22 = {1: P2_WIDE}
            for i in range(nit):
                chunk_layer(bbf[:, C_W21 : C_W21 + H], csl(hT_all, i), bias(1),
                            csl(x21a, i), offload=OFF21.get(i))
                chunk_layer(bbf[:, C_W22 : C_W22 + H], csl(hT_all, i), bias(2),
                            csl(x22a, i), offload=OFF22.get(i))

            flush_chain(all_=True)
            OFFM1 = {0: P2_NARROW, 3: P2_NARROW}
            OFFM2 = {1: P2_NARROW}
            psH = {}
            for i in range(nit):
                chunk_layer(bbf[:, C_WM1 : C_WM1 + H], csl(x21a, i), bias(3),
                            csl(x21b, i), offload=OFFM1.get(i))
                chunk_layer(bbf[:, C_WM2 : C_WM2 + H], csl(x22a, i), bias(4),
                            csl(x22b, i), offload=OFFM2.get(i))
                if i == 5:
                    flush_chain(all_=True)
                    # half 0 (chunks 0-5) complete: heads + QP under the
                    # remaining drains; arithmetic on the idle Pool engine.
                    psH[0] = ppool.tile([128, SC], FP, tag="ps", name="psH0")
                    heads31(0, psH[0])
                    heads32(0, psH[0])
                    qp_post_a(0, psH[0], G)
                    qp_post_b(0, psH[0], G)
            flush_chain(all_=True)
            psH[1] = ppool.tile([128, SC], FP, tag="ps", name="psH1")
            heads31(1, psH[1])
            qp_post_a(1, psH[1], V)
            heads32(1, psH[1])
            qp_post_b(1, psH[1], V)

    nc.compile()
    return nc


def _get_nc(ns=NS):
    if ns not in _cache:
        _cache[ns] = build(ns)
    return _cache[ns]


def prep_maps(inputs, ns=NS, n_cores=N_CORES):
    """Host-side shard + layout prep. Returns per-core in_maps."""
    f32 = np.float32
    g = {k: np.asarray(v) for k, v in inputs.items()}
    x = np.ascontiguousarray(g["x"], f32)
    mean = np.asarray(g["mean"], f32)
    std = np.asarray(g["std"], f32)
    obs = np.array([10.0, 0.0, 10.0, 0.0, 9.0, 0.0], f32)
    moff = mean - obs
    perm = [0, 2, 4, 1, 3, 5]  # pos-block | vel-block order
    qcs = []
    for _, jh in HALVES:
        qcs.append(np.concatenate([
            np.repeat(std[perm], jh),
            np.repeat(moff[perm], jh),
            np.repeat(np.asarray(g["fc31_b"], f32), jh),
            np.repeat(np.asarray(g["fc32_b"], f32), jh),
        ]))
    qc = np.concatenate(qcs)
    bfp = np.concatenate([
        np.broadcast_to(qc, (H, qc.size)),
        np.asarray(g["fc1_b"], f32)[:, None],
        np.asarray(g["fc21_b"], f32)[:, None],
        np.asarray(g["fc22_b"], f32)[:, None],
        np.asarray(g["fcm1_b"], f32)[:, None],
        np.asarray(g["fcm2_b"], f32)[:, None],
    ], axis=1)
    w1pad = np.zeros((H, H), f32)
    w1pad[:NF, :] = np.asarray(g["fc1_w"], f32).T
    bbf = np.concatenate([
        np.asarray(g["fc21_w"], f32).T,
        np.asarray(g["fc22_w"], f32).T,
        np.asarray(g["fcm1_w"], f32).T,
        np.asarray(g["fcm2_w"], f32).T,
        w1pad,
        np.asarray(g["fc31_w"], f32).T,
        np.asarray(g["fc32_w"], f32).T,
    ], axis=1).astype(BF_NP)

    shared = {
        "bbf": np.ascontiguousarray(bbf),
        "bfp": np.ascontiguousarray(bfp, f32),
    }
    in_maps = []
    for c in range(n_cores):
        sh = x[c * ns : (c + 1) * ns]
        m = dict(shared)
        m["x"] = np.ascontiguousarray(sh)
        m["xt"] = np.ascontiguousarray(sh.T.astype(BF_NP))
        in_maps.append(m)
    return in_maps


def kernel(**inputs):
    nc = _get_nc()
    in_maps = prep_maps(inputs)
    res = bass_utils.run_bass_kernel_spmd(nc, in_maps, core_ids=list(range(N_CORES)))
    return np.concatenate([res.results[c]["u"] for c in range(N_CORES)], axis=0)


# revision 16
# speedup vs baseline: 1.1966x; 1.0147x over previous
"""BarrierNet Trainium2 kernel: MLP (6->128->128x2 branches->heads) + closed-form QP.

Data-parallel over 8 cores (16384 samples each). Host pre-shards and
pre-transposes: xt [6,NS] bf16, and packs all weights/biases/QP constants into
two blob tensors (1 bf16 + 1 fp32) loaded with single DMAs. Per core:
  - MLP in transposed layout (hidden on partitions, batch free), chunks of
    2048; each layer = 4 bf16 N=512 matmuls into a 4-bank PSUM group drained
    by ONE fused bias+tanh ACT op (fp32 PSUM -> bf16 SBUF).
  - The ACT engine is the bottleneck (1 elem/lane/cycle), so ~7 of the 41
    chunk-drains are offloaded to the otherwise-idle DVE as a polynomial
    tanh approximation in bf16 (deg-2 in x^2 for the narrow-range layers,
    a Pade(3,2) rational for fc1). Offloaded chunks sit at phase starts so
    their longer latency hides under the ACT drains of the same phase.
  - Heads: per 128-sample block the activation chunk is the STATIONARY
    matmul operand and the tiny head weight [128,3]/[128,2] is the moving
    one, so head output lands directly in the QP sample-grid layout in PSUM.
  - QP runs batched fp32 in a [128, j] sample grid split into asymmetric
    halves (12288 + 4096 samples): the big half's QP runs on the Pool engine
    under the fcm drains; only the small half's QP is a tail.
"""
import sys

sys.path.insert(0, "/opt/trn_rl_repo")

import numpy as np
import ml_dtypes

import concourse.bacc as bacc
import concourse.bass as bass
import concourse.mybir as mybir
import concourse.tile as tile
from concourse import bass_utils

FP = mybir.dt.float32
BF = mybir.dt.bfloat16
AF = mybir.ActivationFunctionType
OP = mybir.AluOpType
BF_NP = ml_dtypes.bfloat16

N_CORES = 8
B = 131072
NS = B // N_CORES          # samples per core
SC = 2048                  # super-chunk (one PSUM group span)
H = 128
NF = 6

# QP halves: (sample offset, jh = samples per grid partition)
HALVES = [(0, 96), (12288, 32)]
QCB = [0, 17 * 96]         # qc column base per half
BFP_W = 17 * 96 + 17 * 32

# blob_bf columns (bf16)
C_W21, C_W22, C_WM1, C_WM2, C_W1, C_WH1, C_WH2 = 0, 128, 256, 384, 512, 640, 643
BBF_W = 645

# tanh approximations (minimax fits of tanh(x)/x in t=x^2, bf16-validated)
P2_WIDE = (0.95591543, -0.20820148, 0.02276596)    # |x|<=2.0 (fc21/fc22)
P2_NARROW = (0.99716053, -0.30797275, 0.07279147)  # |x|<=1.0 (fcm1/fcm2)

_cache = {}


def build(ns=NS):
    nit = ns // SC
    nc = bacc.Bacc("TRN2", target_bir_lowering=False, debug=False)

    x_d = nc.dram_tensor("x", [ns, NF], FP, kind="ExternalInput")
    xt_d = nc.dram_tensor("xt", [NF, ns], BF, kind="ExternalInput")
    bbf_d = nc.dram_tensor("bbf", [H, BBF_W], BF, kind="ExternalInput")
    bfp_d = nc.dram_tensor("bfp", [H, BFP_W], FP, kind="ExternalInput")
    bb_d = nc.dram_tensor("bb", [H, 5], FP, kind="ExternalInput")
    u_d = nc.dram_tensor("u", [ns, 3], FP, kind="ExternalOutput")

    with tile.TileContext(nc) as tc:
        with (
            tc.tile_pool(name="const", bufs=1) as cpool,
            tc.tile_pool(name="act", bufs=2) as apool,
            tc.tile_pool(name="psum", bufs=2, space="PSUM") as ppool,
            tc.tile_pool(name="qp", bufs=2) as qpool,
            tc.tile_pool(name="xb", bufs=2) as xpool,
        ):
            bbf = cpool.tile([H, BBF_W], BF, tag="bbf", name="bbf")
            nc.sync.dma_start(bbf[:], bbf_d[:])
            bb = cpool.tile([H, 5], FP, tag="bb", name="bb")
            nc.sync.dma_start(bb[:], bb_d[:])
            bfp = cpool.tile([H, BFP_W], FP, tag="bfp", name="bfp")
            nc.sync.dma_start(bfp[:], bfp_d[:])

            def bias(k):
                return bb[:, k : k + 1]

            hT_all = cpool.tile([H, ns], BF, tag="hT_all", name="hT_all")
            x21a = cpool.tile([H, ns], BF, tag="x21a", name="x21a")
            x22a = cpool.tile([H, ns], BF, tag="x22a", name="x22a")
            x22b = cpool.tile([H, ns], BF, tag="x22b", name="x22b")
            # x21b overlays hT_all: hT is fully consumed by fc21/fc22 before
            # fcm1 writes x21b (subtile deps order the overwrite per chunk).
            x21b = hT_all

            # QP scratch: per-half persistent values + one shared transient
            # region (each engine runs its QP ops in-order, so reuse is safe).
            pers = [cpool.tile([H, 7 * jh], FP, tag=f"pers{h}", name=f"pers{h}")
                    for h, (_, jh) in enumerate(HALVES)]
            scr = cpool.tile([H, 1536], FP, tag="qscr", name="qscr")
            # DVE tanh-offload scratch (bf16)
            tt = cpool.tile([H, SC], BF, tag="tt", name="tt")
            th = cpool.tile([H, SC], BF, tag="th", name="th")

            V = nc.vector
            G = nc.gpsimd

            def S(lo, w):
                return scr[:, lo : lo + w]

            # Dummy sigmoid up front: the compiler then picks the activation
            # table set containing BOTH sigmoid and tanh -> one table load.
            V.memset(S(1520, 8), 0.0)
            nc.scalar.activation(S(1528, 8), S(1520, 8), AF.Sigmoid)

            # ---------------- QP (sample-grid layout, fp32, batched) --------
            xgs = {}
            for hh, (off, jh) in enumerate(HALVES):
                xg = qpool.tile([128, 6 * 96], FP, tag="xg", name="xg")
                nc.sync.dma_start(xg[:, 0 : 6 * jh],
                    x_d[off : off + 128 * jh, :].rearrange(
                        "(p j) f -> p (j f)", p=128))
                xgs[hh] = xg

            def qp_pre(hh):
                """x-side preamble: needs only x_d -- runs on idle DVE."""
                off, jh = HALVES[hh]
                J3, J6 = 3 * jh, 6 * jh
                qc0 = QCB[hh]
                xg = xgs[hh]
                xgv = xg[:, 0:J6].rearrange("p (j g e) -> p e g j", g=3, e=2)
                x0 = S(0, J6)
                x0v = x0.rearrange("p (e g j) -> p e g j", e=2, g=3)
                qsv = bfp[:, qc0 : qc0 + J6].rearrange(
                    "p (e g j) -> p e g j", e=2, g=3)
                qmv = bfp[:, qc0 + J6 : qc0 + 2 * J6].rearrange(
                    "p (e g j) -> p e g j", e=2, g=3)
                V.tensor_mul(x0v, xgv, qsv)
                V.tensor_add(x0v, x0v, qmv)
                # Persistent values are pre-scaled so the post chain is pure
                # tensor-tensor ops (the Pool engine's ISA has no
                # tensor-scalar): d3x4 = 4d^3 = -G, bar16 = 16*barrier,
                # bd4 = 4*barrier_dot, Ls12 = Lf2b, rg = 1/GG.
                dd, vv = x0[:, 0:J3], x0[:, J3:J6]
                d3 = pers[hh][:, 0:J3]
                d2 = S(J6, J3); V.tensor_mul(d2, dd, dd)
                V.tensor_mul(d3, d2, dd)
                V.tensor_scalar(d3, d3, 4.0, None, OP.mult)    # d3x4
                d4 = S(0, J3); V.tensor_mul(d4, d2, d2)        # over dd
                v2 = S(J6 + J3, J3); V.tensor_mul(v2, vv, vv)
                dv = S(J6 + 2 * J3, J3); V.tensor_mul(dv, d3, vv)   # 4 d^3 v
                dw = S(J3, J3); V.tensor_mul(dw, d2, v2)       # over vv
                d6 = S(J6, J3); V.tensor_mul(d6, d3, d3)       # 16 d^6, over d2
                g6 = S(J6 + 3 * J3, jh)

                def a3(t, k):
                    return t[:, k * jh : (k + 1) * jh]

                def sum3(t, r, bias_const=None):
                    V.tensor_add(r, a3(t, 0), a3(t, 1))
                    if bias_const is None:
                        V.tensor_add(r, r, a3(t, 2))
                    else:
                        V.scalar_tensor_tensor(r, r, bias_const, a3(t, 2),
                                               OP.add, OP.add)

                bar = pers[hh][:, J3 : J3 + jh]
                bd = pers[hh][:, J3 + jh : J3 + 2 * jh]
                Ls = pers[hh][:, J3 + 2 * jh : J3 + 3 * jh]
                sum3(d4, bar, -2401.0)                 # barrier
                V.tensor_scalar(bar, bar, 16.0, None, OP.mult)  # 16*barrier
                sum3(dv, bd)                           # barrier_dot
                V.tensor_scalar(bd, bd, 4.0, None, OP.mult)     # 4*dot
                sum3(dw, Ls)                           # Lf2b / 12
                V.tensor_scalar(Ls, Ls, 12.0, None, OP.mult)    # Lf2b
                sum3(d6, g6)                           # GG
                V.reciprocal(pers[hh][:, J3 + 3 * jh : J3 + 4 * jh], g6)

            # Stationary-operand views matching the QP grid: grid partition p
            # holds samples off + p*jh + j, so head block j takes columns
            # {off + i*jh + j : i=0..127} (stride jh) of the activation tiles.
            def hview(t, hh):
                off, jh = HALVES[hh]
                return t[:, off : off + 128 * jh].rearrange(
                    "p (i j) -> p j i", j=jh)

            def heads31(hh, psH):
                v = hview(x21b, hh)
                for j in range(HALVES[hh][1]):
                    nc.tensor.matmul(psH[:, 5 * j : 5 * j + 3], v[:, j, :],
                                     bbf[:, C_WH1 : C_WH1 + 3],
                                     start=True, stop=True)

            def heads32(hh, psH):
                v = hview(x22b, hh)
                for j in range(HALVES[hh][1]):
                    nc.tensor.matmul(psH[:, 5 * j + 3 : 5 * j + 5], v[:, j, :],
                                     bbf[:, C_WH2 : C_WH2 + 2],
                                     start=True, stop=True)

            # qp_post scratch offsets (units of jh)
            def qp_post_a(hh, psH, E):
                """x31-side: needs only the fcm1 branch (heads31 results)."""
                off, jh = HALVES[hh]
                J2, J3, J6 = 2 * jh, 3 * jh, 6 * jh
                qc0 = QCB[hh]
                hgv = psH[:, 0 : 5 * jh].rearrange("p (j c) -> p c j", c=5)
                x31v = S(2 * J2, J3)
                V.tensor_add(x31v.rearrange("p (c j) -> p c j", c=3),
                             hgv[:, 0:3, :],
                             bfp[:, qc0 + 2 * J6 : qc0 + 2 * J6 + J3].rearrange(
                                 "p (c j) -> p c j", c=3))
                gx = S(2 * J2 + J3, J3)
                E.tensor_mul(gx, pers[hh][:, 0:J3], x31v)
                gu = S(2 * J2 + 2 * J3, jh)
                E.tensor_add(gu, gx[:, 0:jh], gx[:, jh : 2 * jh])
                E.tensor_add(gu, gu, gx[:, 2 * jh : J3])

            def qp_post_b(hh, psH, E):
                off, jh = HALVES[hh]
                J2, J3, J6 = 2 * jh, 3 * jh, 6 * jh
                qc0 = QCB[hh]
                p_ = pers[hh]
                bar = p_[:, J3 : J3 + jh]
                bd = p_[:, J3 + jh : J3 + 2 * jh]
                Ls = p_[:, J3 + 2 * jh : J3 + 3 * jh]
                rg = p_[:, J3 + 3 * jh : J3 + 4 * jh]
                hgv = psH[:, 0 : 5 * jh].rearrange("p (j c) -> p c j", c=5)
                zs = S(0, J2)
                V.tensor_add(zs.rearrange("p (c j) -> p c j", c=2),
                             hgv[:, 3:5, :],
                             bfp[:, qc0 + 2 * J6 + J3 : qc0 + 17 * jh].rearrange(
                                 "p (c j) -> p c j", c=2))
                sg = S(J2, J2)
                nc.scalar.activation(sg, zs, AF.Sigmoid)
                x31v = S(2 * J2, J3)
                gu = S(2 * J2 + 2 * J3, jh)
                o = 2 * J2 + 2 * J3 + jh
                gxb = 2 * J2 + J3    # gx region, dead after gu
                s0t, s1t = sg[:, 0:jh], sg[:, jh:J2]
                # x32sum*bdot = 4*sigmoid_sum*bdot = ssum*bd4;
                # x32prod*bar = 16*sigmoid_prod*bar = sprod*bar16.
                ssum = S(o, jh); E.tensor_add(ssum, s0t, s1t)
                sprod = S(o + jh, jh); E.tensor_mul(sprod, s0t, s1t)
                t1 = S(o + 2 * jh, jh); E.tensor_mul(t1, ssum, bd)
                t2 = S(o + 3 * jh, jh); E.tensor_mul(t2, sprod, bar)
                qa = S(gxb, jh); E.tensor_sub(qa, gu, t1)
                qb = S(gxb + jh, jh); E.tensor_add(qb, Ls, t2)
                q = S(gxb + 2 * jh, jh); E.tensor_sub(q, qa, qb)
                E.tensor_relu(q, q)
                lam = S(o, jh); E.tensor_mul(lam, q, rg)    # over ssum
                ui = qpool.tile([128, 3 * 96], FP, tag="ui", name="ui")
                uiv = ui[:, 0:J3].rearrange("p (j c) -> p c j", c=3)
                w = S(o + jh, jh)                           # over sprod
                for a in range(3):
                    # u = lam*4d^3 - x31  (G = -4d^3, d3 holds 4d^3)
                    E.tensor_mul(w, lam, p_[:, a * jh : (a + 1) * jh])
                    E.tensor_sub(uiv[:, a, :], w,
                                 x31v[:, a * jh : (a + 1) * jh])
                nc.sync.dma_start(
                    u_d[off : off + 128 * jh, :].rearrange(
                        "(p j) c -> p (j c)", p=128),
                    ui[:, 0:J3])

            # ---------------- MLP: layer-outer phases ----------------
            pending = []   # deferred DVE tanh chains (bounds xb tiles at 2)

            def flush_chain(all_=False):
                while pending and (all_ or len(pending) >= 2):
                    pending.pop(0)()

            def dve_tanh(ps, w, bias_t, out_sl, kind):
                """Drain a PSUM group via DVE tanh approximation (bf16).
                The PSUM->bf16 copy (emitted now, freeing the PSUM group
                quickly) folds in the per-channel bias; the arithmetic chain
                is deferred so later copies are not stuck behind it."""
                xb = xpool.tile([H, SC], BF, tag="txb", name="txb")
                V.tensor_scalar(xb[:, 0:w], ps[:, 0:w], bias_t, None, OP.add)

                def chain():
                    t, h, xbs = tt[:, 0:w], th[:, 0:w], xb[:, 0:w]
                    V.tensor_mul(t, xbs, xbs)
                    if kind == "pade":
                        # x*(t+27)/(9t+27), clamp
                        V.tensor_scalar(h, t, 27.0, None, OP.add)
                        V.tensor_mul(h, h, xbs)
                        V.tensor_scalar(t, t, 9.0, 27.0, OP.mult, OP.add)
                        with nc.allow_low_precision(reason="pade recip bf16"):
                            V.reciprocal(t, t)
                        V.tensor_mul(out_sl, h, t)
                    else:
                        c0, c1, c2 = kind
                        V.tensor_scalar(h, t, c2, c1, OP.mult, OP.add)
                        V.tensor_mul(h, h, t)
                        V.tensor_scalar(h, h, c0, None, OP.add)
                        V.tensor_mul(out_sl, h, xbs)
                    V.tensor_scalar(out_sl, out_sl, 1.0, -1.0, OP.min, OP.max)

                pending.append(chain)
                flush_chain()

            def chunk_layer(lhsT, rhs_sl, bias_t, out_sl, w=SC, offload=None):
                ps = ppool.tile([128, SC], FP, tag="ps", name="ps")
                for m in range((w + 511) // 512):
                    mw = min(512, w - 512 * m)
                    nc.tensor.matmul(
                        ps[:, 512 * m : 512 * m + mw],
                        lhsT,
                        rhs_sl[:, 512 * m : 512 * m + mw],
                        start=True, stop=True,
                    )
                if offload is None:
                    nc.scalar.activation(out_sl, ps[:, 0:w], AF.Tanh,
                                         bias=bias_t)
                else:
                    dve_tanh(ps, w, bias_t, out_sl, offload)

            def csl(t, i, w=SC):
                return t[:, SC * i : SC * i + w]

            # fc1 phase: first chunk split 512+1536 for a faster ACT start.
            # fc1 bias is NOT folded into the matmul, so add it via the ACT
            # op (bias arg) or, for the offloaded chunk, fold into xt? --
            # offloaded fc1 chunk handles bias by a pre-add on DVE instead.
            w1 = bbf[0:NF, C_W1 : C_W1 + H]
            for i in range(nit):
                xt_c = apool.tile([NF, SC], BF, tag="xt_c", name="xt_c")
                eng = nc.gpsimd.dma_start if i % 2 == 0 else nc.sync.dma_start
                eng(xt_c[:], xt_d[:, SC * i : SC * (i + 1)])
                if i == 0:
                    chunk_layer(w1, xt_c[:, 0:512], bias(0), csl(hT_all, 0, 512),
                                w=512)
                    chunk_layer(w1, xt_c[:, 512:2048], bias(0),
                                hT_all[:, 512:2048], w=1536)
                else:
                    chunk_layer(w1, xt_c[:], bias(0), csl(hT_all, i))
            flush_chain(all_=True)
            qp_pre(0)
            qp_pre(1)

            OFF21 = {0: P2_WIDE, 3: P2_WIDE}
            OFF22 = {1: P2_WIDE}
            for i in range(nit):
                chunk_layer(bbf[:, C_W21 : C_W21 + H], csl(hT_all, i), bias(1),
                            csl(x21a, i), offload=OFF21.get(i))
                chunk_layer(bbf[:, C_W22 : C_W22 + H], csl(hT_all, i), bias(2),
                            csl(x22a, i), offload=OFF22.get(i))

            flush_chain(all_=True)
            OFFM1 = {0: P2_NARROW, 3: P2_NARROW}
            OFFM2 = {1: P2_NARROW}
            psH = {}
            for i in range(nit):
                chunk_layer(bbf[:, C_WM1 : C_WM1 + H], csl(x21a, i), bias(3),
                            csl(x21b, i), offload=OFFM1.get(i))
                chunk_layer(bbf[:, C_WM2 : C_WM2 + H], csl(x22a, i), bias(4),
                            csl(x22b, i), offload=OFFM2.get(i))
                if i == 5:
                    flush_chain(all_=True)
                    # half 0 (chunks 0-5) complete: heads + QP under the
                    # remaining drains; arithmetic on the idle Pool engine.
                    psH[0] = ppool.tile([128, SC], FP, tag="ps", name="psH0")
                    heads31(0, psH[0])
                    heads32(0, psH[0])
                    qp_post_a(0, psH[0], G)
                    qp_post_b(0, psH[0], G)
            flush_chain(all_=True)
            psH[1] = ppool.tile([128, SC], FP, tag="ps", name="psH1")
            heads31(1, psH[1])
            qp_post_a(1, psH[1], V)
            heads32(1, psH[1])
            qp_post_b(1, psH[1], V)

    nc.compile()
    return nc


def _get_nc(ns=NS):
    if ns not in _cache:
        _cache[ns] = build(ns)
    return _cache[ns]


def prep_maps(inputs, ns=NS, n_cores=N_CORES):
    """Host-side shard + layout prep. Returns per-core in_maps."""
    f32 = np.float32
    g = {k: np.asarray(v) for k, v in inputs.items()}
    x = np.ascontiguousarray(g["x"], f32)
    mean = np.asarray(g["mean"], f32)
    std = np.asarray(g["std"], f32)
    obs = np.array([10.0, 0.0, 10.0, 0.0, 9.0, 0.0], f32)
    moff = mean - obs
    perm = [0, 2, 4, 1, 3, 5]  # pos-block | vel-block order
    qcs = []
    for _, jh in HALVES:
        qcs.append(np.concatenate([
            np.repeat(std[perm], jh),
            np.repeat(moff[perm], jh),
            np.repeat(np.asarray(g["fc31_b"], f32), jh),
            np.repeat(np.asarray(g["fc32_b"], f32), jh),
        ]))
    qc = np.concatenate(qcs)
    bfp = np.broadcast_to(qc, (H, qc.size))
    bb = np.stack([np.asarray(g[k], f32) for k in
                   ("fc1_b", "fc21_b", "fc22_b", "fcm1_b", "fcm2_b")], axis=1)
    w1pad = np.zeros((H, H), f32)
    w1pad[:NF, :] = np.asarray(g["fc1_w"], f32).T
    bbf = np.concatenate([
        np.asarray(g["fc21_w"], f32).T,
        np.asarray(g["fc22_w"], f32).T,
        np.asarray(g["fcm1_w"], f32).T,
        np.asarray(g["fcm2_w"], f32).T,
        w1pad,
        np.asarray(g["fc31_w"], f32).T,
        np.asarray(g["fc32_w"], f32).T,
    ], axis=1).astype(BF_NP)

    shared = {
        "bbf": np.ascontiguousarray(bbf),
        "bfp": np.ascontiguousarray(bfp, f32),
        "bb": np.ascontiguousarray(bb, f32),
    }
    in_maps = []
    for c in range(n_cores):
        sh = x[c * ns : (c + 1) * ns]
        m = dict(shared)
        m["x"] = np.ascontiguousarray(sh)
        m["xt"] = np.ascontiguousarray(sh.T.astype(BF_NP))
        in_maps.append(m)
    return in_maps


def kernel(**inputs):
    nc = _get_nc()
    in_maps = prep_maps(inputs)
    res = bass_utils.run_bass_kernel_spmd(nc, in_maps, core_ids=list(range(N_CORES)))
    return np.concatenate([res.results[c]["u"] for c in range(N_CORES)], axis=0)


# revision 18
# speedup vs baseline: 1.2544x; 1.0483x over previous
"""BarrierNet Trainium2 kernel: MLP (6->128->128x2 branches->heads) + closed-form QP.

Data-parallel over 8 cores (16384 samples each). Host pre-shards and
pre-transposes: xt [6,NS] bf16, and packs all weights/biases/QP constants into
two blob tensors (1 bf16 + 1 fp32) loaded with single DMAs. Per core:
  - MLP in transposed layout (hidden on partitions, batch free), chunks of
    2048; each layer = 4 bf16 N=512 matmuls into a 4-bank PSUM group drained
    by ONE fused bias+tanh ACT op (fp32 PSUM -> bf16 SBUF).
  - The ACT engine is the bottleneck (1 elem/lane/cycle), so ~7 of the 41
    chunk-drains are offloaded to the otherwise-idle DVE as a polynomial
    tanh approximation in bf16 (deg-2 in x^2 for the narrow-range layers,
    a Pade(3,2) rational for fc1). Offloaded chunks sit at phase starts so
    their longer latency hides under the ACT drains of the same phase.
  - Heads: per 128-sample block the activation chunk is the STATIONARY
    matmul operand and the tiny head weight [128,3]/[128,2] is the moving
    one, so head output lands directly in the QP sample-grid layout in PSUM.
  - QP runs batched fp32 in a [128, j] sample grid split into asymmetric
    halves (12288 + 4096 samples): the big half's QP runs on the Pool engine
    under the fcm drains; only the small half's QP is a tail.
"""
import sys

sys.path.insert(0, "/opt/trn_rl_repo")

import numpy as np
import ml_dtypes

import concourse.bacc as bacc
import concourse.bass as bass
import concourse.mybir as mybir
import concourse.tile as tile
from concourse import bass_utils

FP = mybir.dt.float32
BF = mybir.dt.bfloat16
AF = mybir.ActivationFunctionType
OP = mybir.AluOpType
BF_NP = ml_dtypes.bfloat16

N_CORES = 8
B = 131072
NS = B // N_CORES          # samples per core
SC = 2048                  # super-chunk (one PSUM group span)
H = 128
NF = 6

# QP halves: (sample offset, jh = samples per grid partition)
HALVES = [(0, 96), (12288, 32)]
QCB = [0, 17 * 96]         # qc column base per half
BFP_W = 17 * 96 + 17 * 32 + 5  # last 5: fp32 biases for DVE path

# blob_bf columns (bf16)
C_W21, C_W22, C_WM1, C_WM2, C_W1, C_WH1, C_WH2 = 0, 128, 256, 384, 512, 640, 643
BBF_W = 650  # last 5 cols: biases (bf16)

# tanh approximations (minimax fits of tanh(x)/x in t=x^2, bf16-validated)
P2_WIDE = (0.95591543, -0.20820148, 0.02276596)    # |x|<=2.0 (fc21/fc22)
P2_NARROW = (0.99716053, -0.30797275, 0.07279147)  # |x|<=1.0 (fcm1/fcm2)

_cache = {}


def build(ns=NS):
    nit = ns // SC
    nc = bacc.Bacc("TRN2", target_bir_lowering=False, debug=False)

    x_d = nc.dram_tensor("x", [ns, NF], FP, kind="ExternalInput")
    xt_d = nc.dram_tensor("xt", [NF, ns], BF, kind="ExternalInput")
    bbf_d = nc.dram_tensor("bbf", [H, BBF_W], BF, kind="ExternalInput")
    bfp_d = nc.dram_tensor("bfp", [H, BFP_W], FP, kind="ExternalInput")
    u_d = nc.dram_tensor("u", [ns, 3], FP, kind="ExternalOutput")

    with tile.TileContext(nc) as tc:
        with (
            tc.tile_pool(name="const", bufs=1) as cpool,
            tc.tile_pool(name="act", bufs=2) as apool,
            tc.tile_pool(name="psum", bufs=2, space="PSUM") as ppool,
            tc.tile_pool(name="qp", bufs=2) as qpool,
            tc.tile_pool(name="xb", bufs=2) as xpool,
        ):
            bbf = cpool.tile([H, BBF_W], BF, tag="bbf", name="bbf")
            nc.sync.dma_start(bbf[:], bbf_d[:])
            bfp = cpool.tile([H, BFP_W], FP, tag="bfp", name="bfp")
            nc.sync.dma_start(bfp[:], bfp_d[:])

            def bias(k):
                return bbf[:, 645 + k : 646 + k]

            def bias32(k):
                return bfp[:, BFP_W - 5 + k : BFP_W - 4 + k]

            hT_all = cpool.tile([H, ns], BF, tag="hT_all", name="hT_all")
            x21a = cpool.tile([H, ns], BF, tag="x21a", name="x21a")
            x22a = cpool.tile([H, ns], BF, tag="x22a", name="x22a")
            x22b = cpool.tile([H, ns], BF, tag="x22b", name="x22b")
            # x21b overlays hT_all: hT is fully consumed by fc21/fc22 before
            # fcm1 writes x21b (subtile deps order the overwrite per chunk).
            x21b = hT_all

            # QP scratch: per-half persistent values + one shared transient
            # region (each engine runs its QP ops in-order, so reuse is safe).
            pers = [cpool.tile([H, 7 * jh], FP, tag=f"pers{h}", name=f"pers{h}")
                    for h, (_, jh) in enumerate(HALVES)]
            scr = cpool.tile([H, 1536], FP, tag="qscr", name="qscr")
            # DVE tanh-offload scratch (bf16)
            tt = cpool.tile([H, SC], BF, tag="tt", name="tt")
            th = cpool.tile([H, SC], BF, tag="th", name="th")

            V = nc.vector
            G = nc.gpsimd

            def S(lo, w):
                return scr[:, lo : lo + w]

            # Dummy sigmoid up front: the compiler then picks the activation
            # table set containing BOTH sigmoid and tanh -> one table load.
            V.memset(S(1520, 8), 0.0)
            nc.scalar.activation(S(1528, 8), S(1520, 8), AF.Sigmoid)

            # ---------------- QP (sample-grid layout, fp32, batched) --------

            def qp_pre(hh):
                """x-side preamble: needs only x_d -- runs on idle DVE."""
                off, jh = HALVES[hh]
                J3, J6 = 3 * jh, 6 * jh
                qc0 = QCB[hh]
                xg = xgs[hh]
                xgv = xg[:, 0:J6].rearrange("p (j g e) -> p e g j", g=3, e=2)
                x0 = S(0, J6)
                x0v = x0.rearrange("p (e g j) -> p e g j", e=2, g=3)
                qsv = bfp[:, qc0 : qc0 + J6].rearrange(
                    "p (e g j) -> p e g j", e=2, g=3)
                qmv = bfp[:, qc0 + J6 : qc0 + 2 * J6].rearrange(
                    "p (e g j) -> p e g j", e=2, g=3)
                V.tensor_mul(x0v, xgv, qsv)
                V.tensor_add(x0v, x0v, qmv)
                # Persistent values are pre-scaled so the post chain is pure
                # tensor-tensor ops (the Pool engine's ISA has no
                # tensor-scalar): d3x4 = 4d^3 = -G, bar16 = 16*barrier,
                # bd4 = 4*barrier_dot, Ls12 = Lf2b, rg = 1/GG.
                dd, vv = x0[:, 0:J3], x0[:, J3:J6]
                d3 = pers[hh][:, 0:J3]
                d2 = S(J6, J3); V.tensor_mul(d2, dd, dd)
                V.tensor_mul(d3, d2, dd)
                V.tensor_scalar(d3, d3, 4.0, None, OP.mult)    # d3x4
                d4 = S(0, J3); V.tensor_mul(d4, d2, d2)        # over dd
                v2 = S(J6 + J3, J3); V.tensor_mul(v2, vv, vv)
                dv = S(J6 + 2 * J3, J3); V.tensor_mul(dv, d3, vv)   # 4 d^3 v
                dw = S(J3, J3); V.tensor_mul(dw, d2, v2)       # over vv
                d6 = S(J6, J3); V.tensor_mul(d6, d3, d3)       # 16 d^6, over d2
                g6 = S(J6 + 3 * J3, jh)

                def a3(t, k):
                    return t[:, k * jh : (k + 1) * jh]

                def sum3(t, r, bias_const=None):
                    V.tensor_add(r, a3(t, 0), a3(t, 1))
                    if bias_const is None:
                        V.tensor_add(r, r, a3(t, 2))
                    else:
                        V.scalar_tensor_tensor(r, r, bias_const, a3(t, 2),
                                               OP.add, OP.add)

                bar = pers[hh][:, J3 : J3 + jh]
                bd = pers[hh][:, J3 + jh : J3 + 2 * jh]
                Ls = pers[hh][:, J3 + 2 * jh : J3 + 3 * jh]
                sum3(d4, bar, -2401.0)                 # barrier
                V.tensor_scalar(bar, bar, 16.0, None, OP.mult)  # 16*barrier
                sum3(dv, bd)                           # barrier_dot
                V.tensor_scalar(bd, bd, 4.0, None, OP.mult)     # 4*dot
                sum3(dw, Ls)                           # Lf2b / 12
                V.tensor_scalar(Ls, Ls, 12.0, None, OP.mult)    # Lf2b
                sum3(d6, g6)                           # GG
                V.reciprocal(pers[hh][:, J3 + 3 * jh : J3 + 4 * jh], g6)

            # Stationary-operand views matching the QP grid: grid partition p
            # holds samples off + p*jh + j, so head block j takes columns
            # {off + i*jh + j : i=0..127} (stride jh) of the activation tiles.
            def hview(t, hh):
                off, jh = HALVES[hh]
                return t[:, off : off + 128 * jh].rearrange(
                    "p (i j) -> p j i", j=jh)

            def heads31(hh, psH):
                v = hview(x21b, hh)
                for j in range(HALVES[hh][1]):
                    nc.tensor.matmul(psH[:, 5 * j : 5 * j + 3], v[:, j, :],
                                     bbf[:, C_WH1 : C_WH1 + 3],
                                     start=True, stop=True)

            def heads32(hh, psH):
                v = hview(x22b, hh)
                for j in range(HALVES[hh][1]):
                    nc.tensor.matmul(psH[:, 5 * j + 3 : 5 * j + 5], v[:, j, :],
                                     bbf[:, C_WH2 : C_WH2 + 2],
                                     start=True, stop=True)

            # qp_post scratch offsets (units of jh)
            def qp_post_a(hh, psH, E):
                """x31-side: needs only the fcm1 branch (heads31 results)."""
                off, jh = HALVES[hh]
                J2, J3, J6 = 2 * jh, 3 * jh, 6 * jh
                qc0 = QCB[hh]
                hgv = psH[:, 0 : 5 * jh].rearrange("p (j c) -> p c j", c=5)
                x31v = S(2 * J2, J3)
                V.tensor_add(x31v.rearrange("p (c j) -> p c j", c=3),
                             hgv[:, 0:3, :],
                             bfp[:, qc0 + 2 * J6 : qc0 + 2 * J6 + J3].rearrange(
                                 "p (c j) -> p c j", c=3))
                gx = S(2 * J2 + J3, J3)
                E.tensor_mul(gx, pers[hh][:, 0:J3], x31v)
                gu = S(2 * J2 + 2 * J3, jh)
                E.tensor_add(gu, gx[:, 0:jh], gx[:, jh : 2 * jh])
                E.tensor_add(gu, gu, gx[:, 2 * jh : J3])

            def qp_post_b(hh, psH, E):
                off, jh = HALVES[hh]
                J2, J3, J6 = 2 * jh, 3 * jh, 6 * jh
                qc0 = QCB[hh]
                p_ = pers[hh]
                bar = p_[:, J3 : J3 + jh]
                bd = p_[:, J3 + jh : J3 + 2 * jh]
                Ls = p_[:, J3 + 2 * jh : J3 + 3 * jh]
                rg = p_[:, J3 + 3 * jh : J3 + 4 * jh]
                hgv = psH[:, 0 : 5 * jh].rearrange("p (j c) -> p c j", c=5)
                zs = S(0, J2)
                V.tensor_add(zs.rearrange("p (c j) -> p c j", c=2),
                             hgv[:, 3:5, :],
                             bfp[:, qc0 + 2 * J6 + J3 : qc0 + 17 * jh].rearrange(
                                 "p (c j) -> p c j", c=2))
                sg = S(J2, J2)
                nc.scalar.activation(sg, zs, AF.Sigmoid)
                x31v = S(2 * J2, J3)
                gu = S(2 * J2 + 2 * J3, jh)
                o = 2 * J2 + 2 * J3 + jh
                gxb = 2 * J2 + J3    # gx region, dead after gu
                s0t, s1t = sg[:, 0:jh], sg[:, jh:J2]
                # x32sum*bdot = 4*sigmoid_sum*bdot = ssum*bd4;
                # x32prod*bar = 16*sigmoid_prod*bar = sprod*bar16.
                ssum = S(o, jh); E.tensor_add(ssum, s0t, s1t)
                sprod = S(o + jh, jh); E.tensor_mul(sprod, s0t, s1t)
                t1 = S(o + 2 * jh, jh); E.tensor_mul(t1, ssum, bd)
                t2 = S(o + 3 * jh, jh); E.tensor_mul(t2, sprod, bar)
                qa = S(gxb, jh); E.tensor_sub(qa, gu, t1)
                qb = S(gxb + jh, jh); E.tensor_add(qb, Ls, t2)
                q = S(gxb + 2 * jh, jh); E.tensor_sub(q, qa, qb)
                E.tensor_relu(q, q)
                lam = S(o, jh); E.tensor_mul(lam, q, rg)    # over ssum
                ui = qpool.tile([128, 3 * 96], FP, tag="ui", name="ui")
                uiv = ui[:, 0:J3].rearrange("p (j c) -> p c j", c=3)
                w = S(o + jh, jh)                           # over sprod
                for a in range(3):
                    # u = lam*4d^3 - x31  (G = -4d^3, d3 holds 4d^3)
                    E.tensor_mul(w, lam, p_[:, a * jh : (a + 1) * jh])
                    E.tensor_sub(uiv[:, a, :], w,
                                 x31v[:, a * jh : (a + 1) * jh])
                nc.sync.dma_start(
                    u_d[off : off + 128 * jh, :].rearrange(
                        "(p j) c -> p (j c)", p=128),
                    ui[:, 0:J3])

            # ---------------- MLP: layer-outer phases ----------------
            pending = []   # deferred DVE tanh chains (bounds xb tiles at 2)

            def flush_chain(all_=False):
                while pending and (all_ or len(pending) >= 2):
                    pending.pop(0)()

            def dve_tanh(ps, w, bias_t, out_sl, kind):
                """Drain a PSUM group via DVE tanh approximation (bf16).
                The PSUM->bf16 copy (emitted now, freeing the PSUM group
                quickly) folds in the per-channel bias; the arithmetic chain
                is deferred so later copies are not stuck behind it."""
                xb = xpool.tile([H, SC], BF, tag="txb", name="txb")
                V.tensor_scalar(xb[:, 0:w], ps[:, 0:w], bias_t, None, OP.add)

                def chain():
                    t, h, xbs = tt[:, 0:w], th[:, 0:w], xb[:, 0:w]
                    V.tensor_mul(t, xbs, xbs)
                    if kind == "pade":
                        # x*(t+27)/(9t+27), clamp
                        V.tensor_scalar(h, t, 27.0, None, OP.add)
                        V.tensor_mul(h, h, xbs)
                        V.tensor_scalar(t, t, 9.0, 27.0, OP.mult, OP.add)
                        with nc.allow_low_precision(reason="pade recip bf16"):
                            V.reciprocal(t, t)
                        V.tensor_mul(out_sl, h, t)
                    else:
                        c0, c1, c2 = kind
                        V.tensor_scalar(h, t, c2, c1, OP.mult, OP.add)
                        V.tensor_mul(h, h, t)
                        V.tensor_scalar(h, h, c0, None, OP.add)
                        V.tensor_mul(out_sl, h, xbs)
                    V.tensor_scalar(out_sl, out_sl, 1.0, -1.0, OP.min, OP.max)

                pending.append(chain)
                flush_chain()

            def chunk_layer(lhsT, rhs_sl, bias_t, out_sl, w=SC, offload=None,
                            bias_f32=None):
                ps = ppool.tile([128, SC], FP, tag="ps", name="ps")
                for m in range((w + 511) // 512):
                    mw = min(512, w - 512 * m)
                    nc.tensor.matmul(
                        ps[:, 512 * m : 512 * m + mw],
                        lhsT,
                        rhs_sl[:, 512 * m : 512 * m + mw],
                        start=True, stop=True,
                    )
                if offload is None:
                    nc.scalar.activation(out_sl, ps[:, 0:w], AF.Tanh,
                                         bias=bias_t)
                else:
                    dve_tanh(ps, w, bias_f32, out_sl, offload)

            def csl(t, i, w=SC):
                return t[:, SC * i : SC * i + w]

            # fc1 phase: first chunk split 512+1536 for a faster ACT start.
            # fc1 bias is NOT folded into the matmul, so add it via the ACT
            # op (bias arg) or, for the offloaded chunk, fold into xt? --
            # offloaded fc1 chunk handles bias by a pre-add on DVE instead.
            w1 = bbf[0:NF, C_W1 : C_W1 + H]
            xgs = {}
            for i in range(nit):
                xt_c = apool.tile([NF, SC], BF, tag="xt_c", name="xt_c")
                nc.gpsimd.dma_start(xt_c[:], xt_d[:, SC * i : SC * (i + 1)])
                if i == 0:
                    # xg loads ride the Pool SWDGE queue right behind xt0
                    for hh, (off, jh) in enumerate(HALVES):
                        xg = qpool.tile([128, 6 * 96], FP, tag="xg", name="xg")
                        nc.gpsimd.dma_start(xg[:, 0 : 6 * jh],
                            x_d[off : off + 128 * jh, :].rearrange(
                                "(p j) f -> p (j f)", p=128))
                        xgs[hh] = xg
                if i == 0:
                    chunk_layer(w1, xt_c[:, 0:512], bias(0), csl(hT_all, 0, 512),
                                w=512)
                    chunk_layer(w1, xt_c[:, 512:2048], bias(0),
                                hT_all[:, 512:2048], w=1536)
                else:
                    chunk_layer(w1, xt_c[:], bias(0), csl(hT_all, i))
            flush_chain(all_=True)
            qp_pre(0)
            qp_pre(1)

            OFF21 = {0: P2_WIDE, 3: P2_WIDE}
            OFF22 = {1: P2_WIDE}
            for i in range(nit):
                chunk_layer(bbf[:, C_W21 : C_W21 + H], csl(hT_all, i), bias(1),
                            csl(x21a, i), offload=OFF21.get(i),
                            bias_f32=bias32(1))
                chunk_layer(bbf[:, C_W22 : C_W22 + H], csl(hT_all, i), bias(2),
                            csl(x22a, i), offload=OFF22.get(i),
                            bias_f32=bias32(2))

            flush_chain(all_=True)
            OFFM1 = {0: P2_NARROW}
            OFFM2 = {1: P2_NARROW}
            psH = {}
            for i in range(nit):
                chunk_layer(bbf[:, C_WM1 : C_WM1 + H], csl(x21a, i), bias(3),
                            csl(x21b, i), offload=OFFM1.get(i),
                            bias_f32=bias32(3))
                chunk_layer(bbf[:, C_WM2 : C_WM2 + H], csl(x22a, i), bias(4),
                            csl(x22b, i), offload=OFFM2.get(i),
                            bias_f32=bias32(4))
                if i == 5:
                    flush_chain(all_=True)
                    # half 0 (chunks 0-5) complete: heads + QP under the
                    # remaining drains; arithmetic on the idle Pool engine.
                    psH[0] = ppool.tile([128, SC], FP, tag="ps", name="psH0")
                    heads31(0, psH[0])
                    heads32(0, psH[0])
                    qp_post_a(0, psH[0], G)
                    qp_post_b(0, psH[0], G)
            flush_chain(all_=True)
            psH[1] = ppool.tile([128, SC], FP, tag="ps", name="psH1")
            heads31(1, psH[1])
            qp_post_a(1, psH[1], V)
            heads32(1, psH[1])
            qp_post_b(1, psH[1], V)

    nc.compile()
    return nc


def _get_nc(ns=NS):
    if ns not in _cache:
        _cache[ns] = build(ns)
    return _cache[ns]


def prep_maps(inputs, ns=NS, n_cores=N_CORES):
    """Host-side shard + layout prep. Returns per-core in_maps."""
    f32 = np.float32
    g = {k: np.asarray(v) for k, v in inputs.items()}
    x = np.ascontiguousarray(g["x"], f32)
    mean = np.asarray(g["mean"], f32)
    std = np.asarray(g["std"], f32)
    obs = np.array([10.0, 0.0, 10.0, 0.0, 9.0, 0.0], f32)
    moff = mean - obs
    perm = [0, 2, 4, 1, 3, 5]  # pos-block | vel-block order
    qcs = []
    for _, jh in HALVES:
        qcs.append(np.concatenate([
            np.repeat(std[perm], jh),
            np.repeat(moff[perm], jh),
            np.repeat(np.asarray(g["fc31_b"], f32), jh),
            np.repeat(np.asarray(g["fc32_b"], f32), jh),
        ]))
    qc = np.concatenate(qcs)
    bfp = np.concatenate([
        np.broadcast_to(qc, (H, qc.size)),
        np.stack([np.asarray(g[k], f32) for k in
                  ("fc1_b", "fc21_b", "fc22_b", "fcm1_b", "fcm2_b")], axis=1),
    ], axis=1)
    w1pad = np.zeros((H, H), f32)
    w1pad[:NF, :] = np.asarray(g["fc1_w"], f32).T
    bbf = np.concatenate([
        np.asarray(g["fc21_w"], f32).T,
        np.asarray(g["fc22_w"], f32).T,
        np.asarray(g["fcm1_w"], f32).T,
        np.asarray(g["fcm2_w"], f32).T,
        w1pad,
        np.asarray(g["fc31_w"], f32).T,
        np.asarray(g["fc32_w"], f32).T,
        np.stack([np.asarray(g[k], f32) for k in
                  ("fc1_b", "fc21_b", "fc22_b", "fcm1_b", "fcm2_b")], axis=1),
    ], axis=1).astype(BF_NP)

    shared = {
        "bbf": np.ascontiguousarray(bbf),
        "bfp": np.ascontiguousarray(bfp, f32),
    }
    in_maps = []
    for c in range(n_cores):
        sh = x[c * ns : (c + 1) * ns]
        m = dict(shared)
        m["x"] = np.ascontiguousarray(sh)
        m["xt"] = np.ascontiguousarray(sh.T.astype(BF_NP))
        in_maps.append(m)
    return in_maps


def kernel(**inputs):
    nc = _get_nc()
    in_maps = prep_maps(inputs)
    res = bass_utils.run_bass_kernel_spmd(nc, in_maps, core_ids=list(range(N_CORES)))
    return np.concatenate([res.results[c]["u"] for c in range(N_CORES)], axis=0)
